# revision 1
# baseline (speedup 1.0000x reference)
"""Trainium2 Bass kernel for nn_Attention: per-head QKV attention + out-proj.

Contract: kernel(**inputs) takes FULL unsharded inputs
  x [8, 1024, 768] f32, Wqkv [12, 768, 192] f32, bqkv [12, 192] f32,
  Wo [768, 768] f32, bo [768] f32
returns FULL output [8, 1024, 768] f32.

Strategy: pure data-parallel over batch (8 batches -> 8 NeuronCores), no
collectives.  Each core computes its batch end-to-end in bf16 matmuls.

v2 changes vs v1:
  - x is transposed + cast to bf16 on HOST: device loads xT [768, 1024]
    directly (2KB contiguous rows), no PE transposes, no DVE casts, and
    the first QKV matmul can start as soon as the first kc-chunk lands.
  - weights are host-packed per head-pair so each pair's [768, 256] qk
    block is one 128-descriptor DMA; pair 0 arrives first and unblocks
    the projection pipeline ~15us earlier.
  - softmax denominator reciprocal is broadcast across partitions with a
    tiny K=2 block-diagonal ones matmul (both heads at once) instead of
    a DRAM round-trip: kills the per-pair DMA bounce and the 6.3us tail
    stall, and takes DMA queues out of the main loop entirely.
  - ScalarE runs ONLY the 96 exps (the 101us floor); all DMA triggers
    moved to sync, all elementwise to DVE/GpSimd.
  - qkv/v filler matmuls are emitted in <=6-MM pieces between scores/pv
    chunks so the exp stream never waits behind a long PE blob.
"""

import math
import os
from collections import deque

import numpy as np
import ml_dtypes

import concourse.bass as bass
import concourse.tile as tile
from concourse import bacc, mybir
from concourse.bass_utils import run_bass_kernel_spmd
from concourse.tile_rust import add_dep_helper

B, S, D, H, HD = 8, 1024, 768, 12, 64
SCALE = 1.0 / math.sqrt(D)
FP = mybir.dt.float32
BF = mybir.dt.bfloat16
KC = D // 128   # 6 contraction chunks
SC = S // 128   # 8 seq chunks
NQ = S // 512   # 2 free-dim chunks of 512
NP = H // 2     # 6 head pairs

AluOp = mybir.AluOpType
ActFn = mybir.ActivationFunctionType

# Results of the last hardware run (for test harness introspection).
last_results = None


def _build_kernel_body(tc, out_d, xt_d, wqkp_d, wvp_d, wop_d, bqk_d, bo2_d):
    nc = tc.nc

    # Chain every TensorE instruction to the previous one with a no-sync
    # ordering edge so the Tile scheduler preserves the deliberate
    # scores/pv/filler interleave on the in-order PE stream.
    _pe_last = [None]

    def _chain(inst):
        if _pe_last[0] is not None:
            add_dep_helper(inst.ins, _pe_last[0].ins, sync=False,
                           reason="pe-order")
        _pe_last[0] = inst
        return inst

    def MM(*a, reuse_w=False, **k):
        inst = nc.tensor.matmul(*a, **k)
        if reuse_w:
            inst.ins.ldweights = False
        return _chain(inst)

    from contextlib import ExitStack

    with ExitStack() as ctx:
        wpool = ctx.enter_context(tc.tile_pool(name="weights", bufs=1))
        bigs = ctx.enter_context(tc.tile_pool(name="bigs", bufs=1))
        workp = ctx.enter_context(tc.tile_pool(name="work", bufs=1))
        outp = ctx.enter_context(tc.tile_pool(name="outstage", bufs=2))
        etp = ctx.enter_context(tc.tile_pool(name="et", bufs=4))
        psq = ctx.enter_context(tc.tile_pool(name="ps_t", bufs=2, space="PSUM"))
        pspv = ctx.enter_context(tc.tile_pool(name="ps_pv", bufs=2, space="PSUM"))

        # ---- persistent sbuf tensors ----
        xT = bigs.tile([128, KC, S], BF)
        wqk_sb = [wpool.tile([128, KC, 256], BF, name=f"wqk_{t}")
                  for t in range(NP)]
        wv_sb = wpool.tile([128, KC, D], BF)
        wo_sb = wpool.tile([128, KC, D], BF)
        bqk_sb = wpool.tile([128, 2 * KC], FP)
        bo_sb = wpool.tile([128, D], FP)
        qkT = bigs.tile([128, 2 * KC, S], BF)
        vaug = bigs.tile([128, SC, H * (HD + 1)], BF)
        vaug4 = vaug.rearrange("p s (h c) -> p s h c", c=HD + 1)
        outT = bigs.tile([128, KC, S], BF)
        # block-diagonal ones for the per-pair reciprocal broadcast: rows
        # 0:64 of the bcast output get the recip on partition 0 (head h0),
        # rows 64:128 get the recip on partition 64 (head h1).  Engine
        # writes must start at partition 0/32/64/96, hence the K=65 shape
        # with zero rows 1..63 (and zeroed rbf rows so 0*0 contributes 0).
        ones65 = wpool.tile([65, 128], BF)
        rbf = bigs.tile([65, S], BF, name="rbf")

        nc.gpsimd.memset(ones65[:], 0.0)
        nc.gpsimd.memset(ones65[0:1, 0:64], 1.0)
        nc.gpsimd.memset(ones65[64:65, 64:128], 1.0)
        nc.gpsimd.memset(rbf[:], 0.0)
        nc.gpsimd.memset(vaug4[:, :, :, HD:HD + 1], 1.0)

        # ---- DMAs, need-ordered, all on the sync queue trigger ----
        # DMA queues process ~1 descriptor / 100ns, so every load is split
        # to <=64 descriptors and issued in need order: pair-0 qk weights
        # and xT kc0 first (gating the first scores), then the rest.
        def dma_xt(kc, nsplit):
            step = 128 // nsplit
            for i in range(nsplit):
                p0, p1 = i * step, (i + 1) * step
                nc.sync.dma_start(xT[p0:p1, kc, :],
                                  xt_d[kc * 128 + p0:kc * 128 + p1, :])

        def dma_wqk(t, nsplit):
            w2 = wqkp_d[t * 128:(t + 1) * 128, :].rearrange(
                "p (kc f) -> p kc f", kc=KC)
            step = 128 // nsplit
            for i in range(nsplit):
                p0, p1 = i * step, (i + 1) * step
                nc.sync.dma_start(wqk_sb[t][p0:p1, :, :], w2[p0:p1, :, :])

        dma_xt(0, 4)
        dma_wqk(0, 4)
        for kc in range(1, KC):
            dma_xt(kc, 2)
        bqk2 = bqk_d.rearrange("(p j) -> p j", p=128)
        for i in range(4):
            p0, p1 = i * 32, (i + 1) * 32
            nc.sync.dma_start(bqk_sb[p0:p1, :], bqk2[p0:p1, :])
        for half in range(2):
            p0, p1 = half * 64, (half + 1) * 64
            nc.sync.dma_start(wv_sb[p0:p1, :, :],
                              wvp_d[p0:p1, :].rearrange("p (kc f) -> p kc f",
                                                        kc=KC))
        for t in range(1, NP):
            dma_wqk(t, 2)
        nc.sync.dma_start(wo_sb[:],
                          wop_d.rearrange("p (kc f) -> p kc f", kc=KC))
        nc.sync.dma_start(
            bo_sb[:],
            bo2_d.rearrange("(a f) -> a f", a=1).partition_broadcast(128),
        )

        # ---- building blocks ----
        def qkv_pieces(t, m):
            """Yield the 2 half-contraction pieces of one qk m-block
            projection (m: 0..5 = q of pair m, 6..11 = k of pair m-6)."""
            pair = t
            qk = 0 if m < KC else 1
            col0 = qk * 128
            ps = psq.tile([128, S], FP, tag="ps", name=f"qk_{m}")

            def piece(k0, k1):
                for kc in range(k0, k1):
                    lhsT = wqk_sb[pair][:, kc, col0:col0 + 128]
                    for n in range(NQ):
                        MM(
                            ps[:, n * 512:(n + 1) * 512],
                            lhsT,
                            xT[:, kc, n * 512:(n + 1) * 512],
                            start=(kc == 0),
                            stop=(kc == KC - 1),
                            reuse_w=(n > 0),
                        )
                if k1 == KC:
                    nc.vector.tensor_scalar_add(qkT[:, m, :], ps[:],
                                                bqk_sb[:, m:m + 1])

            yield lambda: piece(0, 3)
            yield lambda: piece(3, KC)

        def v_pieces(sc):
            """Yield 2 head-column pieces of one v chunk: heads 0-7
            (needed by pv from pair 0) then heads 8-11 (pair 4+)."""
            def piece(half):
                w = 512 if half == 0 else 256
                ps = psq.tile([128, S], FP, tag="ps", name=f"v_{sc}_{half}")
                for kc in range(KC):
                    MM(ps[:, 0:w],
                       xT[:, kc, sc * 128:(sc + 1) * 128],
                       wv_sb[:, kc, half * 512:half * 512 + w],
                       start=(kc == 0), stop=(kc == KC - 1))
                nc.vector.tensor_copy(
                    vaug4[:, sc, half * 8:half * 8 + w // HD, 0:HD],
                    ps[:, 0:w].rearrange("p (h c) -> p h c", c=HD),
                )

            yield lambda: piece(0)
            yield lambda: piece(1)

        def scores_one(t, sk, h01, et_pair):
            ps = psq.tile([128, S], FP, tag="ps", name=f"sc_{t}_{sk}_{h01}")
            lo, hi = h01 * 64, (h01 + 1) * 64
            lhsT = qkT[lo:hi, KC + t, sk * 128:(sk + 1) * 128]
            for n in range(NQ):
                MM(
                    ps[:, n * 512:(n + 1) * 512],
                    lhsT,
                    qkT[lo:hi, t, n * 512:(n + 1) * 512],
                    start=True,
                    stop=True,
                    tile_position=(h01 * 64, 0),
                    reuse_w=(n > 0),
                )
            nc.scalar.activation(
                et_pair[h01][:, sk, :], ps[:], ActFn.Exp, scale=SCALE
            )

        def pv_chunk(t, sk, et_pair, pv_pair):
            for h01 in range(2):
                h = 2 * t + h01
                for n in range(NQ):
                    MM(
                        pv_pair[h01][:, n * 512:(n + 1) * 512],
                        vaug4[:, sk, h, :],
                        et_pair[h01][:, sk, n * 512:(n + 1) * 512],
                        start=(sk == 0),
                        stop=(sk == SC - 1),
                        reuse_w=(n > 0),
                    )

        def pv_finalize_a(t, pv_pair):
            # Evacuate the pv accumulators to SBUF immediately (frees the
            # psum pair for pv(t+1) in ~5us) and start the reciprocal of
            # the softmax denominators (row 64 = the vaug ones-column).
            # One recip covers both heads: denominators are staged on
            # partitions 0 and 64 of a [65, S] tile (rows 1..63 junk,
            # never read); engine writes may only start at partition
            # 0/32/64/96.
            rr = workp.tile([65, S], FP, tag="rr", name=f"rr_{t}")
            nc.gpsimd.memset(rr[:], 1.0)
            nc.vector.tensor_copy(rr[0:1, :], pv_pair[0][HD:HD + 1, :])
            nc.vector.tensor_copy(rr[64:65, :], pv_pair[1][HD:HD + 1, :])
            u2 = workp.tile([128, S], FP, tag="u2", name=f"u_{t}")
            nc.vector.tensor_copy(u2[0:HD, :], pv_pair[0][0:HD, :])
            nc.vector.tensor_copy(u2[64:64 + HD, :], pv_pair[1][0:HD, :])
            us = [u2[0:HD, :], u2[64:64 + HD, :]]
            rc = workp.tile([65, S], FP, tag="rc", name=f"rc_{t}")
            nc.vector.reciprocal_approx_fast(rc[:], rr[:])
            nc.vector.tensor_copy(rbf[0:1, :], rc[0:1, :])
            nc.vector.tensor_copy(rbf[64:65, :], rc[64:65, :])
            return us

        def pv_finalize_b(t, us):
            # Deferred a few slots so the fin_a DVE chain has completed
            # and the bcast matmul never stalls the in-order PE stream.
            bc_ps = psq.tile([128, S], FP, tag="ps", name=f"bc_{t}")
            for n in range(NQ):
                MM(bc_ps[:, n * 512:(n + 1) * 512], ones65[:],
                   rbf[:, n * 512:(n + 1) * 512],
                   start=True, stop=True, reuse_w=(n > 0))
            bc_bf = workp.tile([128, S], BF, tag="bc", name=f"bc_{t}")
            nc.vector.tensor_copy(bc_bf[:], bc_ps[:])
            for h01 in range(2):
                nc.vector.tensor_tensor(
                    outT[h01 * 64:(h01 + 1) * 64, t, :],
                    us[h01],
                    bc_bf[h01 * 64:(h01 + 1) * 64, :],
                    op=AluOp.mult,
                )

        # ---- prologue: pair 0 q/k projections (stream behind xT DMA) ----
        for piece in [p for gen in (qkv_pieces(0, 0), qkv_pieces(0, KC))
                      for p in gen]:
            piece()

        # ---- main pipeline ----
        # Flat pipeline over 48 (pair, sk) chunks: pv(j-2) rides 2 slots
        # behind scores(j); v chunks (pair 0) and next-pair q/k
        # projections are emitted as <=6-MM filler pieces between the
        # scores/pv chunks of each slot.
        et_tiles = {}
        pv_tiles = {}
        filler = []  # (ready_slot, fn) in FIFO order per ready time

        def run_filler(j):
            for i, (rdy, fn) in enumerate(filler):
                if rdy <= j:
                    filler.pop(i)
                    fn()
                    return

        def emit_pv(j):
            t, sk = j // SC, j % SC
            if sk == 0:
                pv_tiles[t] = [
                    pspv.tile([HD + 1, S], FP, tag="pv", name=f"pv_{2 * t + i}")
                    for i in range(2)
                ]
            pv_chunk(t, sk, et_tiles[t], pv_tiles[t])
            if sk == SC - 1:
                us = pv_finalize_a(t, pv_tiles[t])
                cur = (t + 1) * SC + 1
                filler.append((cur + 4,
                               lambda t=t, us=us: pv_finalize_b(t, us)))
                del pv_tiles[t], et_tiles[t]

        NCH = NP * SC
        for j in range(NCH):
            t, sk = j // SC, j % SC
            if sk == 0:
                et_tiles[t] = [
                    etp.tile([128, SC, S], BF, tag="et", name=f"et_{t}_{i}")
                    for i in range(2)
                ]
            if t == 0:
                # v heads 0-7 feed pv(0, sk) at slot sk+3; heads 8-11
                # are first read by pv at pair 4 - defer them a pair
                p1, p2 = v_pieces(sk)
                filler.append((j, p1))
                filler.append((j + 8, p2))
            if t + 1 < NP:
                if sk == 3:
                    filler.extend((j, p) for p in qkv_pieces(t + 1, t + 1))
                elif sk == 4:
                    filler.extend((j, p) for p in qkv_pieces(t + 1, KC + t + 1))
            scores_one(t, sk, 0, et_tiles[t])
            run_filler(j)
            scores_one(t, sk, 1, et_tiles[t])
            run_filler(j)
            if j >= 3:
                emit_pv(j - 3)
            run_filler(j)
        for j in range(NCH, NCH + 3):
            emit_pv(j - 3)
            while [f for f in filler if f[0] <= j]:
                run_filler(j)
        while filler:
            run_filler(10 ** 9)

        # ---- output projection ----
        for sc in range(SC):
            ps = psq.tile([128, S], FP, tag="ps", name=f"o_{sc}")
            for kc in range(KC):
                lhsT = outT[:, kc, sc * 128:(sc + 1) * 128]
                MM(ps[:, 0:512], lhsT, wo_sb[:, kc, 0:512],
                   start=(kc == 0), stop=(kc == KC - 1))
                MM(ps[:, 512:D], lhsT, wo_sb[:, kc, 512:D],
                   start=(kc == 0), stop=(kc == KC - 1), reuse_w=True)
            osb = outp.tile([128, D], FP, tag="osb", name=f"osb_{sc}")
            nc.vector.tensor_tensor(osb[:], ps[:, 0:D], bo_sb[:], op=AluOp.add)
            nc.sync.dma_start(out_d[sc * 128:(sc + 1) * 128, :], osb[:])


def build():
    """Build + compile the per-core Bass module. Returns the Bacc object."""
    nc = bacc.Bacc("TRN2", target_bir_lowering=False, debug=False, num_devices=B)
    xt_d = nc.dram_tensor("xt", [D, S], BF, kind="ExternalInput").ap()
    wqkp_d = nc.dram_tensor("wqkp", [NP * 128, KC * 256], BF,
                            kind="ExternalInput").ap()
    wvp_d = nc.dram_tensor("wvp", [128, KC * D], BF, kind="ExternalInput").ap()
    wop_d = nc.dram_tensor("wop", [128, KC * D], BF, kind="ExternalInput").ap()
    bqk_d = nc.dram_tensor("bqk", [2 * D], FP, kind="ExternalInput").ap()
    bo2_d = nc.dram_tensor("bo2", [D], FP, kind="ExternalInput").ap()
    out_d = nc.dram_tensor("out", [S, D], FP, kind="ExternalOutput").ap()
    with tile.TileContext(nc) as tc:
        _build_kernel_body(tc, out_d, xt_d, wqkp_d, wvp_d, wop_d, bqk_d, bo2_d)
    nc.compile()
    return nc


def prep_weights(Wqkv, bqkv, Wo, bo):
    """Host-side weight packing (numpy only)."""
    bf16 = ml_dtypes.bfloat16
    # Wqkv [H, D, 3*HD] -> Wq_all/Wk_all/Wv_all [D, H*HD]
    Wq = np.transpose(Wqkv[:, :, 0:HD], (1, 0, 2)).reshape(D, D)
    Wk = np.transpose(Wqkv[:, :, HD:2 * HD], (1, 0, 2)).reshape(D, D)
    Wv = np.transpose(Wqkv[:, :, 2 * HD:], (1, 0, 2)).reshape(D, D)
    # pair-major qk blocks: wqkp[t] = [128, KC, 256] with row p holding
    # W rows {kc*128+p} for all kc, cols = [q pair cols | k pair cols]
    wqkp = np.empty((NP, 128, KC, 256), dtype=bf16)
    for t in range(NP):
        blk = np.concatenate(
            [Wq[:, t * 128:(t + 1) * 128], Wk[:, t * 128:(t + 1) * 128]],
            axis=1,
        )  # [D, 256]
        wqkp[t] = blk.reshape(KC, 128, 256).transpose(1, 0, 2).astype(bf16)
    wqkp = wqkp.reshape(NP * 128, KC * 256)
    # per-partition-contiguous v / o weights: row p = [W[kc*128+p, :] for kc]
    wvp = Wv.reshape(KC, 128, D).transpose(1, 0, 2).reshape(128, KC * D)
    wop = Wo.reshape(KC, 128, D).transpose(1, 0, 2).reshape(128, KC * D)
    # biases: q then k, partition-major [p, j] with j = m-block id
    bq = bqkv[:, 0:HD].reshape(D)
    bk = bqkv[:, HD:2 * HD].reshape(D)
    bv = bqkv[:, 2 * HD:].reshape(D)
    bqk = np.concatenate([bq, bk]).reshape(2 * KC, 128).T  # [128, 12]
    bo2 = bo.astype(np.float64) + bv.astype(np.float64) @ Wo.astype(np.float64)
    return {
        "wqkp": np.ascontiguousarray(wqkp),
        "wvp": np.ascontiguousarray(wvp.astype(bf16)),
        "wop": np.ascontiguousarray(wop.astype(bf16)),
        "bqk": np.ascontiguousarray(bqk.reshape(2 * D).astype(np.float32)),
        "bo2": np.ascontiguousarray(bo2.astype(np.float32)),
    }


_nc_cache = None


def kernel(x, Wqkv, bqkv, Wo, bo):
    global _nc_cache, last_results
    if _nc_cache is None:
        _nc_cache = build()
    nc = _nc_cache
    w = prep_weights(np.asarray(Wqkv), np.asarray(bqkv), np.asarray(Wo),
                     np.asarray(bo))
    bf16 = ml_dtypes.bfloat16
    x = np.asarray(x, dtype=np.float32)
    in_maps = [
        {"xt": np.ascontiguousarray(x[i].T.astype(bf16)), **w}
        for i in range(B)
    ]
    res = run_bass_kernel_spmd(
        nc, in_maps, core_ids=list(range(B)),
        trace=bool(os.environ.get("KERNEL_TRACE")),
    )
    last_results = res
    out = np.stack([res.results[i]["out"] for i in range(B)], axis=0)
    return out.astype(np.float32)

